# revision 84
# baseline (speedup 1.0000x reference)
"""Trainium2 Bass kernel for nn_AlignLoss3 (anchor-alignment InfoNCE-style loss).

Math reduction
--------------
reference:
    label = argmax(Y, axis=1)                       # (N,) in 0..6
    A = l2norm(anchors)[label]; B = l2norm(X)
    logits = B @ A.T / tau                          # (N, N)
    loss = mean(logsumexp(logits, 1) - diag(logits))

Since logits[i, j] = B[i] . a_norm[label[j]] / tau depends on j only through
label[j] (7 classes), define S = B @ a_norm.T / tau  (N x 7) and the class
histogram cnt[c] = #{j : label[j] = c}.  Then
    logsumexp(logits[i,:]) = log( sum_c cnt[c] * exp(S[i,c]) )
    diag[i]               = S[i, label[i]]
so the N x N matmul collapses to an N x 7 one: the kernel is memory-bound on
reading X (16 MB).

Sharding (8 cores)
------------------
Row-parallel: core k gets the contiguous row block X[1024k:1024(k+1)] (and the
matching Y rows for diag selection).  Y is small (224 KB) and is replicated to
every core so each computes the identical global histogram locally — cheaper
than a collective.  Each core returns sum over its rows of (lse - diag); the
host sums the 8 partials and divides by N (the unshard step for a sum-sharded
scalar).

Per-core pipeline / engine assignment
-------------------------------------
* X tile j = shard rows {8p + j} at partition p, so one flat DMA of the Y
  shard as [128, 8, 7] aligns row 8p+j's onehot with S tile j for the diag.
* Pool casts each X tile f32->bf16 (measured loss error ~2e-6 rel; norms and
  dots both from bf16 are safe).  PE transposes bf16 chunks into one PSUM
  tile; one DVE 2x-mode copy moves it back; bf16 matmuls (anchors stationary
  side is the moving 7-col operand) accumulate S_raw in fp32 PSUM.
* rsqrt is Exp(-0.5*Ln(ss*tau^2)) so all ACT ops share one activation table
  (natural_log_exp: Ln/Exp/Square/Copy) — a single 1283 ns table load, warmed
  off the critical path by a dummy op at t=0.
* The small anchor/Y-shard loads ride the Pool SWDGE ring so the SP HWDGE
  stream is pure X tiles (the serial DMA span paces the pipeline); the
  epilogue runs in two asymmetric batches (tiles 0..6 as soon as tile 6
  lands, tile 7's short chain alone in the kernel tail, routed through ACT
  so it avoids the saturated DVE stream).
* This walrus build encodes at most ONE sync wait per HW instruction, so
  every op is arranged to have single-engine (or single-semaphore) deps —
  see the DVE "bounce" copies and SplitWaitTileContext, which re-homes the
  exit drain's 12 waits onto dedicated SP nops.
"""

import numpy as np

import concourse.bass as bass
import concourse.tile as tile
from concourse import mybir
from concourse.bass_utils import run_bass_kernel_spmd
from concourse.masks import make_identity

N, D, C = 8192, 512, 7
NCORES = 8
P = 128
RPC = N // NCORES            # rows per core = 1024
JT = RPC // P                # X tiles per core = 8
GF = N // P                  # full-Y rows per partition = 64
TAU = 0.07
F32 = mybir.dt.float32
BF16 = mybir.dt.bfloat16
DCH = D // P                 # d-chunks = 4
AF = mybir.ActivationFunctionType
ALU = mybir.AluOpType
AX = mybir.AxisListType


def _bcast_mid(ap: bass.AP, n: int) -> bass.AP:
    """[P, F] -> [P, n, F] with a 0-stride middle dim."""
    return bass.AP(tensor=ap.tensor, offset=ap.offset, ap=[ap.ap[0], [0, n], ap.ap[1]])


class SplitWaitTileContext(tile.TileContext):
    """TileContext whose exit drain never carries more than one sync wait.

    This container's walrus build rejects any instruction encoding more than
    one sync-wait command.  Tile's exit drain waits on every proc's final
    tick (12+ waits here).  Pre-drain, emit one SP nop per pending wait —
    the SP sequencer is in-order, so by the time the real drain issues, the
    wait clock shows everything observed and the drain gets no waits.
    """

    def _drain_and_barrier(self, tick_clock, wait_clock):
        import bass_rust

        nc = self.nc
        # nops emitted ahead of the drain in the SP stream; the drain's
        # excess waits are re-homed onto them one-per-instruction below
        nops = [nc.sync.nop(nofuse=True, hint=f"split_wait_{i}") for i in range(16)]

        drain_inst = nc.sync.drain()
        wait_clock.add_sem_waits(
            drain_inst.ins,
            bass_rust.ScopedClock({None: tick_clock.global_clock}),
        )
        si = drain_inst.ins.sync_info
        waits = list(si.on_wait) if si is not None else []
        if len(waits) > 1:
            assert len(waits) - 1 <= len(nops), "raise the split-wait nop count"
            si.on_wait = waits[-1:]
            for nop, w in zip(nops, waits[:-1]):
                nop.ins.sync_info = bass_rust.SyncInfo(on_wait=[w], on_update=[])

        nc.all_engine_barrier()
        assert self.sems is not None
        popped = nc._tile_sem_poison_stack.pop()
        assert popped is self._sem_poison
        nc.clear_and_free_semaphores(list(self.sems.allocated().values()))
        nc.all_engine_barrier()


def build_kernel() -> bass.Bass:
    nc = bass.Bass()

    xs = nc.dram_tensor("xs", [RPC, D], F32, kind="ExternalInput")
    ys = nc.dram_tensor("ys", [RPC, C], F32, kind="ExternalInput")
    yf = nc.dram_tensor("yf", [N, C], F32, kind="ExternalInput")
    anc = nc.dram_tensor("anc", [C, D], F32, kind="ExternalInput")
    out = nc.dram_tensor("out", [1, 1], F32, kind="ExternalOutput")

    with SplitWaitTileContext(nc) as tc:
        with (
            tc.tile_pool(name="consts", bufs=1) as consts,
            tc.tile_pool(name="xpool", bufs=8) as xpool,
            tc.tile_pool(name="xbpool", bufs=8) as xbpool,
            tc.tile_pool(name="xtpool", bufs=8) as xtpool,
            tc.tile_pool(name="work", bufs=2) as work,
            tc.tile_pool(name="small", bufs=4) as small,
            tc.tile_pool(name="psum", bufs=4, space="PSUM") as psum,
            tc.tile_pool(name="psum_s", bufs=3, space="PSUM") as psum_s,
        ):
            ident_bf = consts.tile([P, P], BF16)
            make_identity(nc, ident_bf[:])
            ones = consts.tile([P, 1], F32)
            nc.vector.memset(ones[:], 1.0)
            ones_r = consts.tile([1, P], F32)
            nc.vector.memset(ones_r[:], 1.0)
            # warm the Ln/Exp/Square/Copy activation table off the critical
            # path (the first table-based ACT op pays a 1283 ns table load)
            warm = consts.tile([1, 1], F32)
            nc.scalar.activation(out=warm[:], in_=ones[:1, :], func=AF.Ln)

            # ---- anchors: rows * (1/(|a|*tau)), cast bf16, transpose ----
            # small loads go on the Pool SWDGE ring so the SP HWDGE stream is
            # pure X tiles (the serial DMA span gates the whole pipeline)
            anc_s = consts.tile([C, D], F32)
            nc.gpsimd.dma_start(out=anc_s[:], in_=anc[:])
            a_scr = consts.tile([C, D], F32)
            a_ss = consts.tile([C, 1], F32)
            nc.scalar.activation(
                out=a_scr[:], in_=anc_s[:], func=AF.Square, accum_out=a_ss[:]
            )
            a_ln = consts.tile([C, 1], F32)
            # exp(-0.5*ln(ss*tau^2)) = 1/(sqrt(ss)*tau)
            nc.scalar.activation(out=a_ln[:], in_=a_ss[:], func=AF.Ln, scale=TAU * TAU)
            a_scl = consts.tile([C, 1], F32)
            nc.scalar.activation(out=a_scl[:], in_=a_ln[:], func=AF.Exp, scale=-0.5)
            # bounce both operands through DVE so the scale mult's deps are
            # all same-engine (one consolidated sync wait)
            a_scl_d = consts.tile([C, 1], F32)
            nc.vector.tensor_copy(out=a_scl_d[:], in_=a_scl[:])
            anc_c = consts.tile([C, D], F32)
            nc.vector.tensor_copy(out=anc_c[:], in_=anc_s[:])
            anc_nb = consts.tile([C, D], BF16)
            nc.vector.tensor_scalar_mul(out=anc_nb[:], in0=anc_c[:], scalar1=a_scl_d[:])
            ancT = consts.tile([P, DCH, C], BF16)
            for t in range(DCH):
                ps_a = psum_s.tile([P, C], BF16, tag="ps_small")
                nc.tensor.transpose(
                    ps_a[:], anc_nb[:, t * P:(t + 1) * P], ident_bf[:C, :C]
                )
                nc.vector.tensor_copy(out=ancT[:, t, :], in_=ps_a[:])

            # ---- global histogram from full (replicated) Y ----
            yf_t = work.tile([P, GF, C], F32)
            nc.sync.dma_start(out=yf_t[:], in_=yf[:].rearrange("(p g) c -> p g c", p=P))
            yf_max = work.tile([P, GF], F32)
            nc.vector.reduce_max(yf_max[:], yf_t[:], axis=AX.X)
            oh_f = work.tile([P, GF, C], F32)
            nc.vector.tensor_tensor(
                out=oh_f[:], in0=yf_t[:],
                in1=yf_max[:].to_broadcast((P, GF, C)), op=ALU.is_ge,
            )
            cnt_pp = small.tile([P, C], F32)
            nc.vector.reduce_sum(
                cnt_pp[:], oh_f[:].rearrange("p g c -> p c g"), axis=AX.X
            )
            # partition-reduce -> [1, 7], then rank-1 broadcast -> [128, 7]
            ps_c = psum_s.tile([1, C], F32, tag="ps_small")
            nc.tensor.matmul(ps_c[:], lhsT=ones[:], rhs=cnt_pp[:], start=True, stop=True)
            cnt_row = small.tile([1, C], F32)
            nc.vector.tensor_copy(out=cnt_row[:], in_=ps_c[:])
            ps_cb = psum_s.tile([P, C], F32, tag="ps_small")
            nc.tensor.matmul(
                ps_cb[:], lhsT=ones_r[:], rhs=cnt_row[:], start=True, stop=True
            )
            cnt_b = consts.tile([P, C], F32)
            nc.vector.tensor_copy(out=cnt_b[:], in_=ps_cb[:])
            # Pool-side copy so Pool epilogue ops see a same-engine operand
            cnt_p = consts.tile([P, C], F32)
            nc.gpsimd.tensor_copy(out=cnt_p[:], in_=cnt_b[:])

            # ---- shard-Y onehot for diag selection ----
            ys_t = work.tile([P, JT, C], F32)
            nc.gpsimd.dma_start(
                out=ys_t[:], in_=ys[:].rearrange("(p j) c -> p j c", p=P)
            )
            ys_max = small.tile([P, JT], F32)
            nc.vector.reduce_max(ys_max[:], ys_t[:], axis=AX.X)
            oh_s = work.tile([P, JT, C], F32)
            nc.vector.tensor_tensor(
                out=oh_s[:], in0=ys_t[:],
                in1=ys_max[:].to_broadcast((P, JT, C)), op=ALU.is_ge,
            )

            ss_all = consts.tile([P, JT], F32)
            S_all = consts.tile([P, JT, C], F32)
            xs_r = xs[:].rearrange("(p j) d -> j p d", j=JT)

            lse_all = small.tile([P, JT], F32)
            diag_all = small.tile([P, JT], F32)

            def epilogue_half(j0: int, nj: int, ps_last=None) -> None:
                js = slice(j0, j0 + nj)
                last = ps_last is not None
                ln_ss = small.tile([P, nj], F32, tag="ln_ss")
                nc.scalar.activation(out=ln_ss[:], in_=ss_all[:, js], func=AF.Ln)
                scl_h = small.tile([P, nj], F32, tag="scl_h")
                nc.scalar.activation(
                    out=scl_h[:], in_=ln_ss[:], func=AF.Exp, scale=-0.5
                )
                expS = small.tile([P, nj, C], F32, tag="expS")
                if last:
                    # nj == 1, so scl is a per-partition scalar: stage S via
                    # a stream-prioritized DVE copy, then fuse the row scale
                    # into the ACT Exp using the DVE-side scl bounce — the
                    # Exp then carries one consolidated DVE wait
                    scl_d = small.tile([P, nj], F32, tag="scl_d")
                    nc.vector.tensor_copy(out=scl_d[:], in_=scl_h[:])
                    S7s = small.tile([P, nj, C], F32, tag="S7s")
                    with tc.high_priority():
                        nc.vector.tensor_copy(out=S7s[:], in_=ps_last[:])
                    nc.scalar.activation(
                        out=expS[:], in_=S7s[:], func=AF.Exp, scale=scl_d[:]
                    )
                else:
                    # bounce both operands into Pool tiles so every op in
                    # this chain has single-semaphore deps, then run the
                    # elementwise work on the tail-idle Pool engine
                    scl_p = small.tile([P, nj], F32, tag="scl_p")
                    nc.gpsimd.tensor_copy(out=scl_p[:], in_=scl_h[:])
                    S_p = small.tile([P, nj, C], F32, tag="S_p")
                    nc.gpsimd.tensor_copy(out=S_p[:], in_=S_all[:, js, :])
                    nc.gpsimd.tensor_tensor(
                        out=S_p[:], in0=S_p[:],
                        in1=scl_p[:].to_broadcast((P, nj, C)), op=ALU.mult,
                    )
                    nc.scalar.activation(out=expS[:], in_=S_p[:], func=AF.Exp)
                zz = small.tile([P, nj, C], F32, tag="zz")
                z_h = small.tile([P, nj], F32, tag="z_h")
                nc.gpsimd.tensor_tensor(
                    out=zz[:], in0=expS[:], in1=_bcast_mid(cnt_p[:], nj), op=ALU.mult
                )
                nc.vector.reduce_sum(z_h[:], zz[:], axis=AX.X)
                nc.scalar.activation(out=lse_all[:, js], in_=z_h[:], func=AF.Ln)

                dd = small.tile([P, nj, C], F32, tag="dd")
                if last:
                    # diag from raw PSUM S (oh_s is an old DVE write, its
                    # wait elides), scaled by the DVE-bounced scl
                    d_raw = small.tile([P, nj], F32, tag="d_raw")
                    nc.vector.tensor_tensor(
                        out=dd[:, 0, :], in0=S7s[:, 0, :],
                        in1=oh_s[:, j0, :], op=ALU.mult,
                    )
                    nc.vector.reduce_sum(
                        d_raw[:], dd[:], axis=AX.X, negate=True
                    )
                    nc.vector.tensor_scalar_mul(
                        out=diag_all[:, js], in0=d_raw[:], scalar1=scl_d[:]
                    )
                else:
                    nc.gpsimd.tensor_tensor(
                        out=dd[:], in0=S_p[:], in1=oh_s[:, js, :], op=ALU.mult
                    )
                    nc.vector.reduce_sum(
                        diag_all[:, js], dd[:], axis=AX.X, negate=True
                    )

            for j in range(JT):
                x_t = xpool.tile([P, D], F32)
                nc.sync.dma_start(out=x_t[:], in_=xs_r[j])

                # Pool: cast to bf16 (the only consumer of the f32 tile)
                xb = xbpool.tile([P, D], BF16)
                nc.gpsimd.tensor_copy(out=xb[:], in_=x_t[:])

                # ACT: row sum of squares (Square is in every activation
                # table — no table swap; single writer engine for ss_all)
                sq_scr = xbpool.tile([P, D], F32, tag="sq_scr")
                nc.scalar.activation(
                    out=sq_scr[:], in_=x_t[:], func=AF.Square,
                    accum_out=ss_all[:, j:j + 1],
                )

                # PE: transpose 4 bf16 chunks into one PSUM tile
                ps_big = psum.tile([P, DCH, P], BF16)
                for t in range(DCH):
                    nc.tensor.transpose(
                        ps_big[:, t, :], xb[:, t * P:(t + 1) * P], ident_bf[:]
                    )
                # one DVE 2x copy PSUM -> SBUF; the last tile's copy is
                # stream-prioritized so it does not queue behind earlier
                # tiles' S copies (it gates the kernel tail)
                xT = xtpool.tile([P, DCH, P], BF16)
                if j == JT - 1:
                    with tc.high_priority():
                        nc.vector.tensor_copy(out=xT[:], in_=ps_big[:])
                else:
                    nc.vector.tensor_copy(out=xT[:], in_=ps_big[:])

                # S_raw[rows, 7] = sum_t xT_t.T @ ancT_t   (anchors carry 1/tau)
                ps_S = psum_s.tile([P, C], F32, tag="ps_small")
                for t in range(DCH):
                    nc.tensor.matmul(
                        ps_S[:], lhsT=xT[:, t, :], rhs=ancT[:, t, :],
                        start=(t == 0), stop=(t == DCH - 1),
                    )
                # stash S_raw per tile (row scale deferred to the epilogue);
                # the last tile's S is consumed straight from PSUM
                if j != JT - 1:
                    nc.vector.tensor_copy(out=S_all[:, j, :], in_=ps_S[:])

                # asymmetric epilogue: tiles 0..6 batched as soon as tile 6
                # completes (hidden under tile 7's stream); only tile 7's
                # short chain sits in the kernel tail
                if j == JT - 2:
                    # hint the scheduler to slot this half's ops ahead of
                    # tile 7's copies in each engine stream (deps still gate)
                    with tc.high_priority():
                        epilogue_half(0, JT - 1)
                elif j == JT - 1:
                    epilogue_half(JT - 1, 1, ps_last=ps_S)

            # ---- final reduction: diag_all is stored negated, so two PE
            # matmuls accumulate sum(lse) - sum(diag) into one PSUM tile
            # (one cross-engine wait each), then a single DVE reduce ----
            ps_f = psum_s.tile([1, JT], F32, tag="ps_small")
            nc.tensor.matmul(ps_f[:], lhsT=ones[:], rhs=lse_all[:], start=True, stop=False)
            nc.tensor.matmul(ps_f[:], lhsT=ones[:], rhs=diag_all[:], start=False, stop=True)
            res = small.tile([1, 1], F32)
            nc.vector.reduce_sum(res[:], ps_f[:], axis=AX.X)
            # out DMA on the ACT HWDGE ring: same-engine ordering with the
            # res copy keeps this at a single sync wait
            nc.scalar.dma_start(out=out[:], in_=res[:])

    return nc


_NC_CACHE: bass.Bass | None = None


def run_with_results(X, Y, anchors, **kwargs):
    """Run on all 8 cores; returns (loss, BassKernelResults)."""
    global _NC_CACHE
    if _NC_CACHE is None:
        _NC_CACHE = build_kernel()
    nc = _NC_CACHE

    X = np.ascontiguousarray(X, dtype=np.float32)
    Y = np.ascontiguousarray(Y, dtype=np.float32)
    anchors = np.ascontiguousarray(anchors, dtype=np.float32)

    in_maps = []
    for k in range(NCORES):
        in_maps.append({
            "xs": X[RPC * k:RPC * (k + 1)],
            "ys": Y[RPC * k:RPC * (k + 1)],
            "yf": Y,
            "anc": anchors,
        })
    res = run_bass_kernel_spmd(nc, in_maps, core_ids=list(range(NCORES)), **kwargs)
    total = np.sum(
        np.array([res.results[k]["out"][0, 0] for k in range(NCORES)], dtype=np.float64)
    )
    return np.float32(total / N), res


def kernel(X: np.ndarray, Y: np.ndarray, anchors: np.ndarray) -> np.ndarray:
    loss, _ = run_with_results(X, Y, anchors)
    return loss


# revision 94
# speedup vs baseline: 1.0362x; 1.0362x over previous
"""Trainium2 Bass kernel for nn_AlignLoss3 (anchor-alignment InfoNCE-style loss).

Math reduction
--------------
reference:
    label = argmax(Y, axis=1)                       # (N,) in 0..6
    A = l2norm(anchors)[label]; B = l2norm(X)
    logits = B @ A.T / tau                          # (N, N)
    loss = mean(logsumexp(logits, 1) - diag(logits))

Since logits[i, j] = B[i] . a_norm[label[j]] / tau depends on j only through
label[j] (7 classes), define S = B @ a_norm.T / tau  (N x 7) and the class
histogram cnt[c] = #{j : label[j] = c}.  Then
    logsumexp(logits[i,:]) = log( sum_c cnt[c] * exp(S[i,c]) )
    diag[i]               = S[i, label[i]]
so the N x N matmul collapses to an N x 7 one: the kernel is memory-bound on
reading X (16 MB).

Sharding (8 cores)
------------------
Row-parallel: core k gets the contiguous row block X[1024k:1024(k+1)] (and the
matching Y rows for diag selection).  Y is small (224 KB) and is replicated to
every core so each computes the identical global histogram locally — cheaper
than a collective.  Each core returns sum over its rows of (lse - diag); the
host sums the 8 partials and divides by N (the unshard step for a sum-sharded
scalar).

Per-core pipeline / engine assignment
-------------------------------------
* X tile j = shard rows {8p + j} at partition p, so one flat DMA of the Y
  shard as [128, 8, 7] aligns row 8p+j's onehot with S tile j for the diag.
* Pool casts each X tile f32->bf16 (measured loss error ~2e-6 rel; norms and
  dots both from bf16 are safe).  PE transposes bf16 chunks into one PSUM
  tile; one DVE 2x-mode copy moves it back; bf16 matmuls (anchors stationary
  side is the moving 7-col operand) accumulate S_raw in fp32 PSUM.
* rsqrt is Exp(-0.5*Ln(ss*tau^2)) so all ACT ops share one activation table
  (natural_log_exp: Ln/Exp/Square/Copy) — a single 1283 ns table load, warmed
  off the critical path by a dummy op at t=0.
* The small anchor/Y-shard loads ride the Pool SWDGE ring so the SP HWDGE
  stream is pure X tiles (the serial DMA span paces the pipeline); the
  epilogue runs in two asymmetric batches (tiles 0..6 as soon as tile 6
  lands, tile 7's short chain alone in the kernel tail, routed through ACT
  so it avoids the saturated DVE stream).
* This walrus build encodes at most ONE sync wait per HW instruction, so
  every op is arranged to have single-engine (or single-semaphore) deps —
  see the DVE "bounce" copies and SplitWaitTileContext, which re-homes the
  exit drain's 12 waits onto dedicated SP nops.
"""

import numpy as np

import concourse.bass as bass
import concourse.tile as tile
from concourse import mybir
from concourse.bass_utils import run_bass_kernel_spmd
from concourse.masks import make_identity

N, D, C = 8192, 512, 7
NCORES = 8
P = 128
RPC = N // NCORES            # rows per core = 1024
JT = RPC // P                # X tiles per core = 8
GF = N // P                  # full-Y rows per partition = 64
TAU = 0.07
F32 = mybir.dt.float32
BF16 = mybir.dt.bfloat16
DCH = D // P                 # d-chunks = 4
AF = mybir.ActivationFunctionType
ALU = mybir.AluOpType
AX = mybir.AxisListType


def _bcast_mid(ap: bass.AP, n: int) -> bass.AP:
    """[P, F] -> [P, n, F] with a 0-stride middle dim."""
    return bass.AP(tensor=ap.tensor, offset=ap.offset, ap=[ap.ap[0], [0, n], ap.ap[1]])


class SplitWaitTileContext(tile.TileContext):
    """TileContext whose exit drain never carries more than one sync wait.

    This container's walrus build rejects any instruction encoding more than
    one sync-wait command.  Tile's exit drain waits on every proc's final
    tick (12+ waits here).  Pre-drain, emit one SP nop per pending wait —
    the SP sequencer is in-order, so by the time the real drain issues, the
    wait clock shows everything observed and the drain gets no waits.
    """

    def _drain_and_barrier(self, tick_clock, wait_clock):
        import bass_rust

        nc = self.nc
        # nops emitted ahead of the drain in the SP stream; the drain's
        # excess waits are re-homed onto them one-per-instruction below
        nops = [nc.sync.nop(nofuse=True, hint=f"split_wait_{i}") for i in range(16)]

        drain_inst = nc.sync.drain()
        wait_clock.add_sem_waits(
            drain_inst.ins,
            bass_rust.ScopedClock({None: tick_clock.global_clock}),
        )
        si = drain_inst.ins.sync_info
        waits = list(si.on_wait) if si is not None else []
        if len(waits) > 1:
            assert len(waits) - 1 <= len(nops), "raise the split-wait nop count"
            si.on_wait = waits[-1:]
            for nop, w in zip(nops, waits[:-1]):
                nop.ins.sync_info = bass_rust.SyncInfo(on_wait=[w], on_update=[])

        nc.all_engine_barrier()
        assert self.sems is not None
        popped = nc._tile_sem_poison_stack.pop()
        assert popped is self._sem_poison
        nc.clear_and_free_semaphores(list(self.sems.allocated().values()))
        nc.all_engine_barrier()


def build_kernel() -> bass.Bass:
    nc = bass.Bass()

    xs = nc.dram_tensor("xs", [RPC, D], F32, kind="ExternalInput")
    ys = nc.dram_tensor("ys", [RPC, C], F32, kind="ExternalInput")
    yf = nc.dram_tensor("yf", [N, C], F32, kind="ExternalInput")
    anc = nc.dram_tensor("anc", [C, D], F32, kind="ExternalInput")
    out = nc.dram_tensor("out", [1, 1], F32, kind="ExternalOutput")

    with SplitWaitTileContext(nc) as tc:
        with (
            tc.tile_pool(name="consts", bufs=1) as consts,
            tc.tile_pool(name="xpool", bufs=8) as xpool,
            tc.tile_pool(name="xbpool", bufs=8) as xbpool,
            tc.tile_pool(name="xtpool", bufs=8) as xtpool,
            tc.tile_pool(name="work", bufs=2) as work,
            tc.tile_pool(name="small", bufs=4) as small,
            tc.tile_pool(name="psum", bufs=4, space="PSUM") as psum,
            tc.tile_pool(name="psum_s", bufs=3, space="PSUM") as psum_s,
        ):
            ident_bf = consts.tile([P, P], BF16)
            make_identity(nc, ident_bf[:])
            ones = consts.tile([P, 1], F32)
            nc.vector.memset(ones[:], 1.0)
            ones_r = consts.tile([1, P], F32)
            nc.vector.memset(ones_r[:], 1.0)
            # warm the Ln/Exp/Square/Copy activation table off the critical
            # path (the first table-based ACT op pays a 1283 ns table load)
            warm = consts.tile([1, 1], F32)
            nc.scalar.activation(out=warm[:], in_=ones[:1, :], func=AF.Ln)

            # ---- anchors: rows * (1/(|a|*tau)), cast bf16, transpose ----
            # small loads go on the Pool SWDGE ring so the SP HWDGE stream is
            # pure X tiles (the serial DMA span gates the whole pipeline)
            anc_s = consts.tile([C, D], F32)
            nc.gpsimd.dma_start(out=anc_s[:], in_=anc[:])
            a_scr = consts.tile([C, D], F32)
            a_ss = consts.tile([C, 1], F32)
            nc.scalar.activation(
                out=a_scr[:], in_=anc_s[:], func=AF.Square, accum_out=a_ss[:]
            )
            a_ln = consts.tile([C, 1], F32)
            # exp(-0.5*ln(ss*tau^2)) = 1/(sqrt(ss)*tau)
            nc.scalar.activation(out=a_ln[:], in_=a_ss[:], func=AF.Ln, scale=TAU * TAU)
            a_scl = consts.tile([C, 1], F32)
            nc.scalar.activation(out=a_scl[:], in_=a_ln[:], func=AF.Exp, scale=-0.5)
            # bounce both operands through Pool so the scale mult's deps
            # are all same-engine (one consolidated sync wait) — and off the
            # saturated DVE stream
            a_scl_p = consts.tile([C, 1], F32)
            nc.gpsimd.tensor_copy(out=a_scl_p[:], in_=a_scl[:])
            anc_p = consts.tile([C, D], F32)
            nc.gpsimd.tensor_copy(out=anc_p[:], in_=anc_s[:])
            anc_nb = consts.tile([C, D], BF16)
            nc.gpsimd.tensor_scalar_mul(out=anc_nb[:], in0=anc_p[:], scalar1=a_scl_p[:])
            ancT = consts.tile([P, DCH, C], BF16)
            for t in range(DCH):
                ps_a = psum_s.tile([P, C], BF16, tag="ps_small")
                nc.tensor.transpose(
                    ps_a[:], anc_nb[:, t * P:(t + 1) * P], ident_bf[:C, :C]
                )
                nc.vector.tensor_copy(out=ancT[:, t, :], in_=ps_a[:])

            # ---- global histogram from full (replicated) Y ----
            yf_t = work.tile([P, GF, C], F32)
            nc.sync.dma_start(out=yf_t[:], in_=yf[:].rearrange("(p g) c -> p g c", p=P))
            yf_max = work.tile([P, GF], F32)
            nc.vector.reduce_max(yf_max[:], yf_t[:], axis=AX.X)
            oh_f = work.tile([P, GF, C], F32)
            nc.vector.tensor_tensor(
                out=oh_f[:], in0=yf_t[:],
                in1=yf_max[:].to_broadcast((P, GF, C)), op=ALU.is_ge,
            )
            cnt_pp = small.tile([P, C], F32)
            nc.vector.reduce_sum(
                cnt_pp[:], oh_f[:].rearrange("p g c -> p c g"), axis=AX.X
            )
            # partition-reduce -> [1, 7], then rank-1 broadcast -> [128, 7]
            ps_c = psum_s.tile([1, C], F32, tag="ps_small")
            nc.tensor.matmul(ps_c[:], lhsT=ones[:], rhs=cnt_pp[:], start=True, stop=True)
            cnt_row = small.tile([1, C], F32)
            nc.vector.tensor_copy(out=cnt_row[:], in_=ps_c[:])
            ps_cb = psum_s.tile([P, C], F32, tag="ps_small")
            nc.tensor.matmul(
                ps_cb[:], lhsT=ones_r[:], rhs=cnt_row[:], start=True, stop=True
            )
            cnt_b = consts.tile([P, C], F32)
            nc.vector.tensor_copy(out=cnt_b[:], in_=ps_cb[:])
            # Pool-side copy so Pool epilogue ops see a same-engine operand
            cnt_p = consts.tile([P, C], F32)
            nc.gpsimd.tensor_copy(out=cnt_p[:], in_=cnt_b[:])

            # ---- shard-Y onehot for diag selection ----
            ys_t = work.tile([P, JT, C], F32)
            nc.gpsimd.dma_start(
                out=ys_t[:], in_=ys[:].rearrange("(p j) c -> p j c", p=P)
            )
            ys_max = small.tile([P, JT], F32)
            nc.vector.reduce_max(ys_max[:], ys_t[:], axis=AX.X)
            oh_s = work.tile([P, JT, C], F32)
            nc.vector.tensor_tensor(
                out=oh_s[:], in0=ys_t[:],
                in1=ys_max[:].to_broadcast((P, JT, C)), op=ALU.is_ge,
            )

            ss_all = consts.tile([P, JT], F32)
            S_all = consts.tile([P, JT, C], F32)
            xs_r = xs[:].rearrange("(p j) d -> j p d", j=JT)

            lse_all = small.tile([P, JT], F32)
            diag_all = small.tile([P, JT], F32)

            def epilogue_half(j0: int, nj: int, ps_last=None) -> None:
                js = slice(j0, j0 + nj)
                last = ps_last is not None
                ln_ss = small.tile([P, nj], F32, tag="ln_ss")
                nc.scalar.activation(out=ln_ss[:], in_=ss_all[:, js], func=AF.Ln)
                scl_h = small.tile([P, nj], F32, tag="scl_h")
                nc.scalar.activation(
                    out=scl_h[:], in_=ln_ss[:], func=AF.Exp, scale=-0.5
                )
                expS = small.tile([P, nj, C], F32, tag="expS")
                if last:
                    # nj == 1, so scl is a per-partition scalar: stage S via
                    # a stream-prioritized DVE copy, then fuse the row scale
                    # into the ACT Exp using the DVE-side scl bounce — the
                    # Exp then carries one consolidated DVE wait
                    scl_d = small.tile([P, nj], F32, tag="scl_d")
                    nc.vector.tensor_copy(out=scl_d[:], in_=scl_h[:])
                    S7s = small.tile([P, nj, C], F32, tag="S7s")
                    with tc.high_priority():
                        nc.vector.tensor_copy(out=S7s[:], in_=ps_last[:])
                    nc.scalar.activation(
                        out=expS[:], in_=S7s[:], func=AF.Exp, scale=scl_d[:]
                    )
                else:
                    # bounce both operands into Pool tiles so every op in
                    # this chain has single-semaphore deps, then run the
                    # elementwise work on the tail-idle Pool engine
                    scl_p = small.tile([P, nj], F32, tag="scl_p")
                    nc.gpsimd.tensor_copy(out=scl_p[:], in_=scl_h[:])
                    S_p = small.tile([P, nj, C], F32, tag="S_p")
                    nc.gpsimd.tensor_copy(out=S_p[:], in_=S_all[:, js, :])
                    nc.gpsimd.tensor_tensor(
                        out=S_p[:], in0=S_p[:],
                        in1=scl_p[:].to_broadcast((P, nj, C)), op=ALU.mult,
                    )
                    nc.scalar.activation(out=expS[:], in_=S_p[:], func=AF.Exp)
                zz = small.tile([P, nj, C], F32, tag="zz")
                z_h = small.tile([P, nj], F32, tag="z_h")
                nc.gpsimd.tensor_tensor(
                    out=zz[:], in0=expS[:], in1=_bcast_mid(cnt_p[:], nj), op=ALU.mult
                )
                nc.vector.reduce_sum(z_h[:], zz[:], axis=AX.X)
                nc.scalar.activation(out=lse_all[:, js], in_=z_h[:], func=AF.Ln)

                dd = small.tile([P, nj, C], F32, tag="dd")
                if last:
                    # diag from raw PSUM S (oh_s is an old DVE write, its
                    # wait elides), scaled by the DVE-bounced scl
                    d_raw = small.tile([P, nj], F32, tag="d_raw")
                    nc.vector.tensor_tensor(
                        out=dd[:, 0, :], in0=S7s[:, 0, :],
                        in1=oh_s[:, j0, :], op=ALU.mult,
                    )
                    nc.vector.reduce_sum(
                        d_raw[:], dd[:], axis=AX.X, negate=True
                    )
                    nc.vector.tensor_scalar_mul(
                        out=diag_all[:, js], in0=d_raw[:], scalar1=scl_d[:]
                    )
                else:
                    nc.gpsimd.tensor_tensor(
                        out=dd[:], in0=S_p[:], in1=oh_s[:, js, :], op=ALU.mult
                    )
                    nc.vector.reduce_sum(
                        diag_all[:, js], dd[:], axis=AX.X, negate=True
                    )

            for j in range(JT):
                x_t = xpool.tile([P, D], F32)
                nc.sync.dma_start(out=x_t[:], in_=xs_r[j])

                # Pool: cast to bf16 (the only consumer of the f32 tile)
                xb = xbpool.tile([P, D], BF16)
                nc.gpsimd.tensor_copy(out=xb[:], in_=x_t[:])

                # ACT: row sum of squares (Square is in every activation
                # table — no table swap; single writer engine for ss_all)
                sq_scr = xbpool.tile([P, D], F32, tag="sq_scr")
                nc.scalar.activation(
                    out=sq_scr[:], in_=x_t[:], func=AF.Square,
                    accum_out=ss_all[:, j:j + 1],
                )

                # PE: transpose 4 bf16 chunks into one PSUM tile
                ps_big = psum.tile([P, DCH, P], BF16)
                for t in range(DCH):
                    nc.tensor.transpose(
                        ps_big[:, t, :], xb[:, t * P:(t + 1) * P], ident_bf[:]
                    )
                # one DVE 2x copy PSUM -> SBUF; the last tile's copy is
                # stream-prioritized so it does not queue behind earlier
                # tiles' S copies (it gates the kernel tail)
                xT = xtpool.tile([P, DCH, P], BF16)
                if j == JT - 1:
                    with tc.high_priority():
                        nc.vector.tensor_copy(out=xT[:], in_=ps_big[:])
                else:
                    nc.vector.tensor_copy(out=xT[:], in_=ps_big[:])

                # S_raw[rows, 7] = sum_t xT_t.T @ ancT_t   (anchors carry 1/tau)
                ps_S = psum_s.tile([P, C], F32, tag="ps_small")
                for t in range(DCH):
                    nc.tensor.matmul(
                        ps_S[:], lhsT=xT[:, t, :], rhs=ancT[:, t, :],
                        start=(t == 0), stop=(t == DCH - 1),
                    )
                # stash S_raw per tile (row scale deferred to the epilogue);
                # the last tile's S is consumed straight from PSUM
                if j != JT - 1:
                    nc.vector.tensor_copy(out=S_all[:, j, :], in_=ps_S[:])

                # asymmetric epilogue: tiles 0..6 batched as soon as tile 6
                # completes (hidden under tile 7's stream); only tile 7's
                # short chain sits in the kernel tail
                if j == JT - 2:
                    # hint the scheduler to slot this half's ops ahead of
                    # tile 7's copies in each engine stream (deps still gate)
                    with tc.high_priority():
                        epilogue_half(0, JT - 1)
                elif j == JT - 1:
                    epilogue_half(JT - 1, 1, ps_last=ps_S)

            # ---- final reduction: diag_all is stored negated, so two PE
            # matmuls accumulate sum(lse) - sum(diag) into one PSUM tile
            # (one cross-engine wait each), then a single DVE reduce ----
            ps_f = psum_s.tile([1, JT], F32, tag="ps_small")
            nc.tensor.matmul(ps_f[:], lhsT=ones[:], rhs=lse_all[:], start=True, stop=False)
            nc.tensor.matmul(ps_f[:], lhsT=ones[:], rhs=diag_all[:], start=False, stop=True)
            res = small.tile([1, 1], F32)
            nc.vector.reduce_sum(res[:], ps_f[:], axis=AX.X)
            # out DMA on the ACT HWDGE ring: same-engine ordering with the
            # res copy keeps this at a single sync wait
            nc.scalar.dma_start(out=out[:], in_=res[:])

    return nc


_NC_CACHE: bass.Bass | None = None


def run_with_results(X, Y, anchors, **kwargs):
    """Run on all 8 cores; returns (loss, BassKernelResults)."""
    global _NC_CACHE
    if _NC_CACHE is None:
        _NC_CACHE = build_kernel()
    nc = _NC_CACHE

    X = np.ascontiguousarray(X, dtype=np.float32)
    Y = np.ascontiguousarray(Y, dtype=np.float32)
    anchors = np.ascontiguousarray(anchors, dtype=np.float32)

    in_maps = []
    for k in range(NCORES):
        in_maps.append({
            "xs": X[RPC * k:RPC * (k + 1)],
            "ys": Y[RPC * k:RPC * (k + 1)],
            "yf": Y,
            "anc": anchors,
        })
    res = run_bass_kernel_spmd(nc, in_maps, core_ids=list(range(NCORES)), **kwargs)
    total = np.sum(
        np.array([res.results[k]["out"][0, 0] for k in range(NCORES)], dtype=np.float64)
    )
    return np.float32(total / N), res


def kernel(X: np.ndarray, Y: np.ndarray, anchors: np.ndarray) -> np.ndarray:
    loss, _ = run_with_results(X, Y, anchors)
    return loss
